# revision 9
# baseline (speedup 1.0000x reference)
"""Trainium2 Bass kernel for the 4-step shift-only MAF (MADE) chain.

Strategy: tensor-parallel over the hidden/feature dims across 8 NeuronCores
(column-parallel for every layer), with activations kept transposed
[features, batch] so matmuls chain without transposes.  The inter-step
`z[:, ::-1]` permute is folded into the host-side weight prep (W0 rows /
W3 cols reversed for odd steps), so the device never flips.  After each
layer an AllGather (partition-axis concat) rebuilds the full activation.

Device per-core program (SPMD, identical for all cores; per-core data
arrives via in_maps):
  z_loc [128,100] and full zT [128,8,100] start as x.
  Per step s: h0 = relu(W0e[s].T @ z) (2 psum m-tiles) -> AG -> h1 -> AG
  -> h2 -> AG -> shift = W3e[s].T @ h2; z_loc -= shift + b3; AG z (not on
  last step).  Finally out = ones.T @ (z_loc^2) per core -> [1,100]; host
  sums the 8 partials and adds the log(2pi) constant.
"""

import os
import sys

import numpy as np

for _p in ("/opt/trn_rl_repo", "/opt/trn_rl_repo/concourse"):
    if _p not in sys.path:
        sys.path.insert(0, _p)

B = 100
DIM = 1024
H = 2048
STEPS = 4
NC = 8
P = 128
KD = DIM // P   # 8 z k-tiles
KH = H // P     # 16 h k-tiles
MH = 2          # h m-tiles per core (256 local cols)
HL = H // NC    # 256
DL = DIM // NC  # 128
LOG_2PI = float(np.log(2.0 * np.pi))
F32 = np.float32

try:
    from ml_dtypes import bfloat16 as BF16
except ImportError:  # pragma: no cover
    BF16 = None

# compute dtype for weights / gathered activations ("float32" | "bfloat16")
WDTYPE = os.environ.get("MAF_WDTYPE", "bfloat16")


def _made_mask(n_in, n_out, exclusive):
    d_in, d_out = n_in // DIM, n_out // DIM
    deg_in = np.arange(n_in) // d_in
    deg_out = np.arange(n_out) // d_out
    if exclusive:
        m = deg_out[None, :] > deg_in[:, None]
    else:
        m = deg_out[None, :] >= deg_in[:, None]
    return m.astype(F32)


def _prep_inputs(x, W0, b0, W1, b1, W2, b2, W3, b3):
    """Host-side: mask, fold flips, shard, pre-arrange into SBUF layouts.

    Returns in_maps: list of dicts, one per core."""
    M0 = _made_mask(DIM, H, True)
    M1 = _made_mask(H, H, False)
    M3 = _made_mask(H, DIM, False)

    xT = np.ascontiguousarray(x.T.astype(F32))              # [1024, 100]
    xt_arr = np.ascontiguousarray(
        xT.reshape(KD, P, B).transpose(1, 0, 2))            # [128, 8, 100]

    # Per-step effective (masked + flip-folded) weights
    W0e, W1e, W2e, W3e, b3e = [], [], [], [], []
    for s in range(STEPS):
        w0 = W0[s] * M0
        if s % 2 == 1:
            w0 = w0[::-1, :]
        w3 = W3[s] * M3
        b3s = b3[s]
        if s % 2 == 1:
            w3 = w3[:, ::-1]
            b3s = b3s[::-1]
        W0e.append(w0)
        W1e.append(W1[s] * M1)
        W2e.append(W2[s] * M1)
        W3e.append(w3)
        b3e.append(b3s)

    wdt = F32 if WDTYPE == "float32" else BF16
    in_maps = []
    for c in range(NC):
        hc = slice(HL * c, HL * (c + 1))
        dc = slice(DL * c, DL * (c + 1))
        w0c = np.stack([
            W0e[s][:, hc].reshape(KD, P, MH, P).transpose(1, 0, 2, 3)
            for s in range(STEPS)])                          # [4,128,8,2,128]
        w1c = np.stack([
            W1e[s][:, hc].reshape(KH, P, MH, P).transpose(1, 0, 2, 3)
            for s in range(STEPS)])                          # [4,128,16,2,128]
        w2c = np.stack([
            W2e[s][:, hc].reshape(KH, P, MH, P).transpose(1, 0, 2, 3)
            for s in range(STEPS)])
        w3c = np.stack([
            W3e[s][:, dc].reshape(KH, P, P).transpose(1, 0, 2)
            for s in range(STEPS)])                          # [4,128,16,128]
        b0c = np.stack([b0[s][hc].reshape(MH, P).T for s in range(STEPS)])
        b1c = np.stack([b1[s][hc].reshape(MH, P).T for s in range(STEPS)])
        b2c = np.stack([b2[s][hc].reshape(MH, P).T for s in range(STEPS)])
        b3c = np.stack([b3e[s][dc].reshape(1, P).T for s in range(STEPS)])
        in_maps.append({
            "xt": np.ascontiguousarray(xt_arr.astype(wdt)),
            "xloc": np.ascontiguousarray(xT[dc, :]),         # [128, 100]
            "w0": np.ascontiguousarray(w0c.astype(wdt)),
            "w1": np.ascontiguousarray(w1c.astype(wdt)),
            "w2": np.ascontiguousarray(w2c.astype(wdt)),
            "w3": np.ascontiguousarray(w3c.astype(wdt)),
            "b0": np.ascontiguousarray(b0c.astype(F32)),
            "b1": np.ascontiguousarray(b1c.astype(F32)),
            "b2": np.ascontiguousarray(b2c.astype(F32)),
            "b3": np.ascontiguousarray(b3c.astype(F32)),
        })
    return in_maps


_CACHED_NC = {}


def _build_module(repeat=1):
    """Build the SPMD module. repeat>1 runs the whole MAF body N times
    back-to-back (timing builds only; output is then meaningless)."""
    if repeat in _CACHED_NC:
        return _CACHED_NC[repeat]

    from concourse import bass, bacc, tile, mybir

    f32 = mybir.dt.float32
    Relu = mybir.ActivationFunctionType.Relu
    Square = mybir.ActivationFunctionType.Square
    RG = [list(range(NC))]

    nc = bacc.Bacc("TRN2", target_bir_lowering=False, debug=False,
                   num_devices=NC)

    xt_d = nc.dram_tensor("xt", [P, KD, B], f32, kind="ExternalInput")
    xloc_d = nc.dram_tensor("xloc", [P, B], f32, kind="ExternalInput")
    w0_d = nc.dram_tensor("w0", [STEPS, P, KD, MH, P], f32, kind="ExternalInput")
    w1_d = nc.dram_tensor("w1", [STEPS, P, KH, MH, P], f32, kind="ExternalInput")
    w2_d = nc.dram_tensor("w2", [STEPS, P, KH, MH, P], f32, kind="ExternalInput")
    w3_d = nc.dram_tensor("w3", [STEPS, P, KH, P], f32, kind="ExternalInput")
    b0_d = nc.dram_tensor("b0", [STEPS, P, MH], f32, kind="ExternalInput")
    b1_d = nc.dram_tensor("b1", [STEPS, P, MH], f32, kind="ExternalInput")
    b2_d = nc.dram_tensor("b2", [STEPS, P, MH], f32, kind="ExternalInput")
    b3_d = nc.dram_tensor("b3", [STEPS, P, 1], f32, kind="ExternalInput")
    sq_d = nc.dram_tensor("sq", [1, B], f32, kind="ExternalOutput")

    with tile.TileContext(nc) as tc:
        with (
            tc.tile_pool(name="w01", bufs=2) as wpool,
            tc.tile_pool(name="hf", bufs=2) as hpool,
            tc.tile_pool(name="zp", bufs=2) as zpool,
            tc.tile_pool(name="loc", bufs=2) as locpool,
            tc.tile_pool(name="bia", bufs=2) as bpool,
            tc.tile_pool(name="cst", bufs=1) as cpool,
            tc.tile_pool(name="ps", bufs=4, space=bass.MemorySpace.PSUM) as pspool,
            tc.tile_pool(name="drb", bufs=2, space="DRAM") as dpool,
        ):
            ones = cpool.tile([P, 1], f32, tag="ones")
            nc.gpsimd.memset(ones[:], 1.0)

            zT = zpool.tile([P, KD, B], f32, tag="zT")
            nc.sync.dma_start(zT[:], xt_d[:])
            zloc = zpool.tile([P, B], f32, tag="zloc")
            nc.sync.dma_start(zloc[:], xloc_d[:])

            def h_layer(w_t, b_t, rhsT, n_k, out_tag):
                """col-parallel hidden layer + AG; returns full hT tile."""
                hloc = locpool.tile([P, MH, B], f32, tag="hloc")
                for m in range(MH):
                    ps = pspool.tile([P, B], f32, tag="ps")
                    for k in range(n_k):
                        nc.tensor.matmul(
                            ps[:], w_t[:, k, m, :], rhsT[:, k, :],
                            start=(k == 0), stop=(k == n_k - 1))
                    nc.scalar.activation(hloc[:, m, :], ps[:], Relu,
                                         bias=b_t[:, m:m + 1], scale=1.0)
                agi = dpool.tile([HL, B], f32, tag="agi")
                nc.sync.dma_start(
                    agi.rearrange("(m p) b -> p m b", p=P), hloc[:])
                ago = dpool.tile([H, B], f32, tag="ago")
                nc.gpsimd.collective_compute(
                    "AllGather", mybir.AluOpType.bypass, replica_groups=RG,
                    ins=[agi.opt()], outs=[ago.opt()])
                hT = hpool.tile([P, KH, B], f32, tag=out_tag)
                nc.sync.dma_start(hT[:], ago.rearrange("(k p) b -> p k b", p=P))
                return hT

            for it in range(STEPS * repeat):
                s = it % STEPS
                is_last = it == STEPS * repeat - 1
                w0t = wpool.tile([P, KD, MH, P], f32, tag="w0")
                nc.sync.dma_start(w0t[:], w0_d[s])
                w1t = wpool.tile([P, KH, MH, P], f32, tag="w1")
                nc.sync.dma_start(w1t[:], w1_d[s])
                w2t = wpool.tile([P, KH, MH, P], f32, tag="w2")
                nc.sync.dma_start(w2t[:], w2_d[s])
                w3t = wpool.tile([P, KH, P], f32, tag="w3")
                nc.sync.dma_start(w3t[:], w3_d[s])
                b0t = bpool.tile([P, MH], f32, tag="b0")
                nc.sync.dma_start(b0t[:], b0_d[s])
                b1t = bpool.tile([P, MH], f32, tag="b1")
                nc.sync.dma_start(b1t[:], b1_d[s])
                b2t = bpool.tile([P, MH], f32, tag="b2")
                nc.sync.dma_start(b2t[:], b2_d[s])
                b3t = bpool.tile([P, 1], f32, tag="b3")
                nc.sync.dma_start(b3t[:], b3_d[s])

                h0T = h_layer(w0t, b0t, zT, KD, "h0T")
                h1T = h_layer(w1t, b1t, h0T, KH, "h1T")
                h2T = h_layer(w2t, b2t, h1T, KH, "h2T")

                ps3 = pspool.tile([P, B], f32, tag="ps")
                for k in range(KH):
                    nc.tensor.matmul(ps3[:], w3t[:, k, :], h2T[:, k, :],
                                     start=(k == 0), stop=(k == KH - 1))
                tmp = locpool.tile([P, B], f32, tag="ztmp")
                nc.vector.tensor_sub(tmp[:], zloc[:], ps3[:])
                zloc2 = zpool.tile([P, B], f32, tag="zloc")
                nc.vector.tensor_scalar_sub(zloc2[:], tmp[:], b3t[:, 0:1])
                zloc = zloc2

                if not is_last:
                    zin = dpool.tile([P, B], f32, tag="zin")
                    nc.sync.dma_start(zin[:], zloc[:])
                    zout = dpool.tile([DIM, B], f32, tag="zout")
                    nc.gpsimd.collective_compute(
                        "AllGather", mybir.AluOpType.bypass,
                        replica_groups=RG,
                        ins=[zin.opt()], outs=[zout.opt()])
                    zT = zpool.tile([P, KD, B], f32, tag="zT")
                    nc.sync.dma_start(
                        zT[:], zout.rearrange("(k p) b -> p k b", p=P))

            z2 = locpool.tile([P, B], f32, tag="z2")
            nc.scalar.activation(z2[:], zloc[:], Square)
            psq = pspool.tile([1, B], f32, tag="psq")
            nc.tensor.matmul(psq[:], ones[:], z2[:], start=True, stop=True)
            sq_sb = locpool.tile([1, B], f32, tag="sqsb")
            nc.vector.tensor_copy(sq_sb[:], psq[:])
            nc.sync.dma_start(sq_d[:], sq_sb[:])

    nc.compile()
    _CACHED_NC[repeat] = nc
    return nc


def kernel(x, W0, b0, W1, b1, W2, b2, W3, b3):
    from concourse import bass_utils

    in_maps = _prep_inputs(x, W0, b0, W1, b1, W2, b2, W3, b3)
    nc = _build_module()
    res = bass_utils.run_bass_kernel_spmd(
        nc, in_maps, core_ids=list(range(NC)),
        trace=bool(int(os.environ.get("MAF_TRACE", "0"))))
    total = np.zeros(B, dtype=np.float64)
    for c in range(NC):
        total += res.results[c]["sq"][0].astype(np.float64)
    out = 0.5 * total + 0.5 * DIM * LOG_2PI
    if res.exec_time_ns is not None:
        kernel.last_exec_time_ns = res.exec_time_ns
    return out.astype(F32)


kernel.last_exec_time_ns = None


# revision 15
# speedup vs baseline: 1.3389x; 1.3389x over previous
"""Trainium2 Bass kernel for the 4-step shift-only MAF (MADE) chain.

Strategy: tensor-parallel over the hidden/feature dims across 8 NeuronCores
(column-parallel for every layer), with activations kept transposed
[features, batch] so matmuls chain without transposes.  The inter-step
`z[:, ::-1]` permute is folded into the host-side weight prep (W0 rows /
W3 cols reversed for odd steps), so the device never flips.  After each
layer an AllGather (partition-axis concat) rebuilds the full activation.

Device per-core program (SPMD, identical for all cores; per-core data
arrives via in_maps):
  z_loc [128,100] and full zT [128,8,100] start as x.
  Per step s: h0 = relu(W0e[s].T @ z) (2 psum m-tiles) -> AG -> h1 -> AG
  -> h2 -> AG -> shift = W3e[s].T @ h2; z_loc -= shift + b3; AG z (not on
  last step).  Finally out = ones.T @ (z_loc^2) per core -> [1,100]; host
  sums the 8 partials and adds the log(2pi) constant.
"""

import os
import sys

import numpy as np

for _p in ("/opt/trn_rl_repo", "/opt/trn_rl_repo/concourse"):
    if _p not in sys.path:
        sys.path.insert(0, _p)

B = 100
DIM = 1024
H = 2048
STEPS = 4
NC = 8
P = 128
KD = DIM // P   # 8 z k-tiles
KH = H // P     # 16 h k-tiles
MH = 2          # h m-tiles per core (256 local cols)
HL = H // NC    # 256
DL = DIM // NC  # 128
LOG_2PI = float(np.log(2.0 * np.pi))
F32 = np.float32

try:
    from ml_dtypes import bfloat16 as BF16
except ImportError:  # pragma: no cover
    BF16 = None

# compute dtype for weights / gathered activations ("float32" | "bfloat16")
WDTYPE = os.environ.get("MAF_WDTYPE", "bfloat16")


def _made_mask(n_in, n_out, exclusive):
    d_in, d_out = n_in // DIM, n_out // DIM
    deg_in = np.arange(n_in) // d_in
    deg_out = np.arange(n_out) // d_out
    if exclusive:
        m = deg_out[None, :] > deg_in[:, None]
    else:
        m = deg_out[None, :] >= deg_in[:, None]
    return m.astype(F32)


def _prep_inputs(x, W0, b0, W1, b1, W2, b2, W3, b3):
    """Host-side: mask, fold flips, shard, pre-arrange into SBUF layouts.

    Returns in_maps: list of dicts, one per core."""
    M0 = _made_mask(DIM, H, True)
    M1 = _made_mask(H, H, False)
    M3 = _made_mask(H, DIM, False)

    xT = np.ascontiguousarray(x.T.astype(F32))              # [1024, 100]
    xt_arr = np.ascontiguousarray(
        xT.reshape(KD, P, B).transpose(1, 0, 2))            # [128, 8, 100]

    # Per-step effective (masked + flip-folded) weights
    W0e, W1e, W2e, W3e, b3e = [], [], [], [], []
    for s in range(STEPS):
        w0 = W0[s] * M0
        if s % 2 == 1:
            w0 = w0[::-1, :]
        w3 = W3[s] * M3
        b3s = b3[s]
        if s % 2 == 1:
            w3 = w3[:, ::-1]
            b3s = b3s[::-1]
        W0e.append(w0)
        W1e.append(W1[s] * M1)
        W2e.append(W2[s] * M1)
        W3e.append(w3)
        b3e.append(b3s)

    wdt = F32 if WDTYPE == "float32" else BF16
    in_maps = []
    for c in range(NC):
        hc = slice(HL * c, HL * (c + 1))
        dc = slice(DL * c, DL * (c + 1))
        w0c = np.stack([
            W0e[s][:, hc].reshape(KD, P, MH, P).transpose(1, 0, 2, 3)
            for s in range(STEPS)])                          # [4,128,8,2,128]
        w1c = np.stack([
            W1e[s][:, hc].reshape(KH, P, MH, P).transpose(1, 0, 2, 3)
            for s in range(STEPS)])                          # [4,128,16,2,128]
        w2c = np.stack([
            W2e[s][:, hc].reshape(KH, P, MH, P).transpose(1, 0, 2, 3)
            for s in range(STEPS)])
        w3c = np.stack([
            W3e[s][:, dc].reshape(KH, P, P).transpose(1, 0, 2)
            for s in range(STEPS)])                          # [4,128,16,128]
        b0c = np.stack([b0[s][hc].reshape(MH, P).T for s in range(STEPS)])
        b1c = np.stack([b1[s][hc].reshape(MH, P).T for s in range(STEPS)])
        b2c = np.stack([b2[s][hc].reshape(MH, P).T for s in range(STEPS)])
        b3c = np.stack([b3e[s][dc].reshape(1, P).T for s in range(STEPS)])
        in_maps.append({
            "xt": np.ascontiguousarray(xt_arr.astype(wdt)),
            "xloc": np.ascontiguousarray(xT[dc, :]),         # [128, 100]
            "w0": np.ascontiguousarray(w0c.astype(wdt)),
            "w1": np.ascontiguousarray(w1c.astype(wdt)),
            "w2": np.ascontiguousarray(w2c.astype(wdt)),
            "w3": np.ascontiguousarray(w3c.astype(wdt)),
            "b0": np.ascontiguousarray(b0c.astype(F32)),
            "b1": np.ascontiguousarray(b1c.astype(F32)),
            "b2": np.ascontiguousarray(b2c.astype(F32)),
            "b3": np.ascontiguousarray(b3c.astype(F32)),
        })
    return in_maps


_CACHED_NC = {}


def _build_module(repeat=1):
    """Build the SPMD module. repeat>1 runs the whole MAF body N times
    back-to-back (timing builds only; output is then meaningless)."""
    if repeat in _CACHED_NC:
        return _CACHED_NC[repeat]

    from concourse import bass, bacc, tile, mybir

    f32 = mybir.dt.float32
    wdt = f32 if WDTYPE == "float32" else mybir.dt.bfloat16
    Relu = mybir.ActivationFunctionType.Relu
    Square = mybir.ActivationFunctionType.Square
    RG = [list(range(NC))]
    no_cc = bool(int(os.environ.get("MAF_NO_CC", "0")))  # timing ablation
    no_w = bool(int(os.environ.get("MAF_NO_W", "0")))    # timing ablation

    nc = bacc.Bacc("TRN2", target_bir_lowering=False, debug=False,
                   num_devices=NC)

    xt_d = nc.dram_tensor("xt", [P, KD, B], wdt, kind="ExternalInput")
    xloc_d = nc.dram_tensor("xloc", [P, B], f32, kind="ExternalInput")
    w0_d = nc.dram_tensor("w0", [STEPS, P, KD, MH, P], wdt, kind="ExternalInput")
    w1_d = nc.dram_tensor("w1", [STEPS, P, KH, MH, P], wdt, kind="ExternalInput")
    w2_d = nc.dram_tensor("w2", [STEPS, P, KH, MH, P], wdt, kind="ExternalInput")
    w3_d = nc.dram_tensor("w3", [STEPS, P, KH, P], wdt, kind="ExternalInput")
    b0_d = nc.dram_tensor("b0", [STEPS, P, MH], f32, kind="ExternalInput")
    b1_d = nc.dram_tensor("b1", [STEPS, P, MH], f32, kind="ExternalInput")
    b2_d = nc.dram_tensor("b2", [STEPS, P, MH], f32, kind="ExternalInput")
    b3_d = nc.dram_tensor("b3", [STEPS, P, 1], f32, kind="ExternalInput")
    sq_d = nc.dram_tensor("sq", [1, B], f32, kind="ExternalOutput")

    with tile.TileContext(nc) as tc:
        with (
            tc.tile_pool(name="w01", bufs=2) as wpool,
            tc.tile_pool(name="hf", bufs=2) as hpool,
            tc.tile_pool(name="zp", bufs=2) as zpool,
            tc.tile_pool(name="loc", bufs=2) as locpool,
            tc.tile_pool(name="bia", bufs=2) as bpool,
            tc.tile_pool(name="cst", bufs=1) as cpool,
            tc.tile_pool(name="ps", bufs=4, space=bass.MemorySpace.PSUM) as pspool,
            tc.tile_pool(name="drb", bufs=2, space="DRAM") as dpool,
        ):
            ones = cpool.tile([P, 1], f32, tag="ones")
            nc.gpsimd.memset(ones[:], 1.0)

            zT = zpool.tile([P, KD, B], wdt, tag="zT")
            nc.sync.dma_start(zT[:], xt_d[:])
            zloc = zpool.tile([P, B], f32, tag="zloc")
            nc.sync.dma_start(zloc[:], xloc_d[:])

            def h_layer(w_t, b_t, rhsT, n_k, out_tag):
                """col-parallel hidden layer + AG; returns full hT tile."""
                hloc = locpool.tile([P, MH, B], wdt, tag="hloc")
                for m in range(MH):
                    ps = pspool.tile([P, B], f32, tag="ps")
                    for k in range(n_k):
                        nc.tensor.matmul(
                            ps[:], w_t[:, k, m, :], rhsT[:, k, :],
                            start=(k == 0), stop=(k == n_k - 1))
                    nc.scalar.activation(hloc[:, m, :], ps[:], Relu,
                                         bias=b_t[:, m:m + 1], scale=1.0)
                agi = dpool.tile([HL, B], wdt, tag="agi")
                nc.sync.dma_start(
                    agi.rearrange("(m p) b -> p m b", p=P), hloc[:])
                ago = dpool.tile([H, B], wdt, tag="ago")
                if no_cc:
                    nc.sync.dma_start(ago[0:HL, :], agi[:])
                else:
                    nc.gpsimd.collective_compute(
                        "AllGather", mybir.AluOpType.bypass, replica_groups=RG,
                        ins=[agi.opt()], outs=[ago.opt()])
                hT = hpool.tile([P, KH, B], wdt, tag=out_tag)
                nc.sync.dma_start(hT[:], ago.rearrange("(k p) b -> p k b", p=P))
                return hT

            for it in range(STEPS * repeat):
                s = it % STEPS
                is_last = it == STEPS * repeat - 1
                w0t = wpool.tile([P, KD, MH, P], wdt, tag="w0")
                nc.sync.dma_start(w0t[:], w0_d[s])
                w1t = wpool.tile([P, KH, MH, P], wdt, tag="w1")
                nc.sync.dma_start(w1t[:], w1_d[s])
                w2t = wpool.tile([P, KH, MH, P], wdt, tag="w2")
                nc.sync.dma_start(w2t[:], w2_d[s])
                w3t = wpool.tile([P, KH, P], wdt, tag="w3")
                nc.sync.dma_start(w3t[:], w3_d[s])
                b0t = bpool.tile([P, MH], f32, tag="b0")
                nc.sync.dma_start(b0t[:], b0_d[s])
                b1t = bpool.tile([P, MH], f32, tag="b1")
                nc.sync.dma_start(b1t[:], b1_d[s])
                b2t = bpool.tile([P, MH], f32, tag="b2")
                nc.sync.dma_start(b2t[:], b2_d[s])
                b3t = bpool.tile([P, 1], f32, tag="b3")
                nc.sync.dma_start(b3t[:], b3_d[s])

                h0T = h_layer(w0t, b0t, zT, KD, "h0T")
                h1T = h_layer(w1t, b1t, h0T, KH, "h1T")
                h2T = h_layer(w2t, b2t, h1T, KH, "h2T")

                ps3 = pspool.tile([P, B], f32, tag="ps")
                for k in range(KH):
                    nc.tensor.matmul(ps3[:], w3t[:, k, :], h2T[:, k, :],
                                     start=(k == 0), stop=(k == KH - 1))
                tmp = locpool.tile([P, B], f32, tag="ztmp")
                nc.vector.tensor_sub(tmp[:], zloc[:], ps3[:])
                zloc2 = zpool.tile([P, B], f32, tag="zloc")
                nc.vector.tensor_scalar_sub(zloc2[:], tmp[:], b3t[:, 0:1])
                zloc = zloc2

                if not is_last:
                    zlb = locpool.tile([P, B], wdt, tag="zlb")
                    nc.vector.tensor_copy(zlb[:], zloc[:])
                    zin = dpool.tile([P, B], wdt, tag="zin")
                    nc.sync.dma_start(zin[:], zlb[:])
                    zout = dpool.tile([DIM, B], wdt, tag="zout")
                    if no_cc:
                        nc.sync.dma_start(zout[0:P, :], zin[:])
                    else:
                        nc.gpsimd.collective_compute(
                            "AllGather", mybir.AluOpType.bypass,
                            replica_groups=RG,
                            ins=[zin.opt()], outs=[zout.opt()])
                    zT = zpool.tile([P, KD, B], wdt, tag="zT")
                    nc.sync.dma_start(
                        zT[:], zout.rearrange("(k p) b -> p k b", p=P))

            z2 = locpool.tile([P, B], f32, tag="z2")
            nc.scalar.activation(z2[:], zloc[:], Square)
            psq = pspool.tile([1, B], f32, tag="psq")
            nc.tensor.matmul(psq[:], ones[:], z2[:], start=True, stop=True)
            sq_sb = locpool.tile([1, B], f32, tag="sqsb")
            nc.vector.tensor_copy(sq_sb[:], psq[:])
            nc.sync.dma_start(sq_d[:], sq_sb[:])

    nc.compile()
    _CACHED_NC[repeat] = nc
    return nc


def kernel(x, W0, b0, W1, b1, W2, b2, W3, b3):
    from concourse import bass_utils

    in_maps = _prep_inputs(x, W0, b0, W1, b1, W2, b2, W3, b3)
    nc = _build_module()
    res = bass_utils.run_bass_kernel_spmd(
        nc, in_maps, core_ids=list(range(NC)),
        trace=bool(int(os.environ.get("MAF_TRACE", "0"))))
    total = np.zeros(B, dtype=np.float64)
    for c in range(NC):
        total += res.results[c]["sq"][0].astype(np.float64)
    out = 0.5 * total + 0.5 * DIM * LOG_2PI
    if res.exec_time_ns is not None:
        kernel.last_exec_time_ns = res.exec_time_ns
    return out.astype(F32)


kernel.last_exec_time_ns = None
